# revision 1
# baseline (speedup 1.0000x reference)
"""Lattice-LSTM NER tagger (nn_BiLSTM_88484916232709) on 8 TRN2 NeuronCores.

Strategy: data-parallel over the batch (B=8 -> one row per core), SPMD (one
program, per-core data). The lattice scan is sequential in T; per step the
word-cell "lattice" edges end at lag d = len-1 in [1, 8], so every edge's
(h, c) source lies in a sliding window of the last 8 states. The kernel bakes
a core-uniform per-step structure: `nb = min(8, j)` base positions (one per
lag, read directly/packed from the state-history window) plus replica
positions when some core has several active edges with the same lag at the
same step (capacity = max over cores). Per-core data (gaz ids, masks) fill
the positions; inactive positions contribute exactly 0 via the mask.

All sigmoids are computed as 0.5*tanh(0.5 x)+0.5 with the affine folded into
pre-scaled weights / fused vector ops so the only ACT tables used are
tanh/exp/identity (one table set -> one ACT_TABLE_LOAD).

Embedding lookups (word/biword/gaz) run on-device via indirect DMA gathers
from the full tables in HBM.
"""

import numpy as np

import concourse.bass as bass
import concourse.mybir as mybir
from concourse.tile import TileContext
from concourse.bass_utils import run_bass_kernel_spmd
from concourse.masks import make_identity

B, T, K, H = 8, 512, 8, 128
DIN, DG, NL = 100, 50, 20
V_WORD, V_BIWORD, V_GAZ = 100000, 200000, 300000
D_WORD, D_BIWORD = 50, 50

F32 = mybir.dt.float32
F16 = mybir.dt.float16
I32 = mybir.dt.int32
AF = mybir.ActivationFunctionType
ALU = mybir.AluOpType
AX = mybir.AxisListType

MMDT = F16  # dtype of recurrent/pre matmul operands (PSUM accum is f32)


def _legalize_single_wait(nc):
    """This walrus build allows at most one sync-wait per instruction.
    Peel extra waits onto same-engine single-wait EventSemaphore insts."""
    k = 0
    for f in nc.m.functions:
        for bb in f.blocks:
            insts = bb.instructions
            i = 0
            while i < len(insts):
                inst = insts[i]
                si = getattr(inst, "sync_info", None)
                if si is not None and len(si.on_wait) > 1:
                    extra = list(si.on_wait[:-1])
                    keep = si.on_wait[-1]
                    peeled = []
                    for w in extra:
                        ev = mybir.InstEventSemaphore(
                            name=f"sw{k}", ins=[], outs=[]
                        )
                        k += 1
                        ev.engine = inst.engine
                        ev.sync_info = mybir.SyncInfo(on_wait=[w], on_update=[])
                        peeled.append(ev)
                    si.on_wait.clear()
                    si.on_wait.append(keep)
                    insts[i:i] = peeled
                    i += len(peeled)
                i += 1
    return k


def build_structure(gaz_starts, gaz_mask, t_run):
    """Core-uniform per-step schedule.

    Returns (steps, NA, NB) where steps[j] is a dict with
      nb, n, off, wordstep, blend, rep_lags (list of lag d per replica,
      ordered), hw_col (blend-mask column or None)
    NA = total packed positions, NB = number of blend steps.
    """
    gs = np.asarray(gaz_starts)
    gm = np.asarray(gaz_mask).astype(bool)
    lag = np.arange(t_run)[None, :, None] - gs[:, :t_run]  # [B,T,K]
    steps = []
    off = 0
    nb_blend = 0
    for j in range(t_run):
        nb = min(8, j)
        act = gm[:, j]  # [B,K]
        d = lag[:, j]  # [B,K]
        counts = np.zeros((B, nb + 1), np.int64)
        for b in range(B):
            for k in range(K):
                if act[b, k]:
                    dd = int(d[b, k])
                    assert 1 <= dd <= nb, (j, b, k, dd, nb)
                    counts[b, dd] += 1
        cap = counts.max(axis=0) if nb > 0 else np.zeros(1, np.int64)
        total = int(counts.sum())
        wordstep = total > 0
        per_core_any = counts.sum(axis=1) > 0
        blend = wordstep and not per_core_any.all()
        rep_lags = []
        if wordstep:
            for dd in range(1, nb + 1):
                for _ in range(max(0, int(cap[dd]) - 1)):
                    rep_lags.append(dd)
        n = (nb + len(rep_lags)) if wordstep else 0
        sd = dict(
            j=j,
            nb=nb,
            n=n,
            off=off,
            wordstep=wordstep,
            blend=blend,
            rep_lags=rep_lags,
            hw_col=nb_blend if blend else None,
        )
        if wordstep:
            off += n
        if blend:
            nb_blend += 1
        steps.append(sd)
    return steps, off, nb_blend


def pack_core(b, steps, gaz_word_ids, gaz_starts, gaz_mask, NA, NB, t_run):
    """Per-core position data: gaz ids, activity mask, has-word blend mask."""
    gid = np.zeros(NA, np.int32)
    msk = np.zeros(NA, np.float32)
    hw = np.zeros(max(NB, 1), np.float32)
    gids = np.asarray(gaz_word_ids)
    gs = np.asarray(gaz_starts)
    gm = np.asarray(gaz_mask).astype(bool)
    for sd in steps:
        j = sd["j"]
        if not sd["wordstep"]:
            continue
        nb, off = sd["nb"], sd["off"]
        by_lag = {}
        for k in range(K):
            if gm[b, j, k]:
                dd = j - int(gs[b, j, k])
                by_lag.setdefault(dd, []).append(int(gids[b, j, k]))
        used = {dd: 0 for dd in by_lag}
        # base positions: q = 0..nb-1 <-> lag nb-q
        for q in range(nb):
            dd = nb - q
            lst = by_lag.get(dd, [])
            if lst:
                gid[off + q] = lst[0]
                msk[off + q] = 1.0
                used[dd] = 1
        # replica positions
        for r, dd in enumerate(sd["rep_lags"]):
            lst = by_lag.get(dd, [])
            u = used.get(dd, 0)
            if len(lst) > u:
                gid[off + nb + r] = lst[u]
                msk[off + nb + r] = 1.0
                used[dd] = u + 1
        if sd["hw_col"] is not None:
            hw[sd["hw_col"]] = 1.0 if by_lag else 0.0
    return gid, msk, hw


def prep_shared(inputs, t_run=T):
    """Host-side shared (core-independent) constant tensors."""
    f = lambda x: np.ascontiguousarray(np.asarray(x, np.float32))
    W_ih, W_hh, b_l = f(inputs["W_ih"]), f(inputs["W_hh"]), f(inputs["b_lstm"])
    Wa_ih, Wa_hh, b_a = f(inputs["Wa_ih"]), f(inputs["Wa_hh"]), f(inputs["b_alpha"])
    Ww_ih, Ww_hh, b_w = f(inputs["Ww_ih"]), f(inputs["Ww_hh"]), f(inputs["b_word"])
    W_tag, b_tag = f(inputs["W_tag"]), f(inputs["b_tag"])

    def gate_scale(WT, scales):  # WT [D, 3H]
        out = WT.copy()
        for g, s in enumerate(scales):
            out[:, g * H:(g + 1) * H] *= s
        return out

    mm = lambda x: np.ascontiguousarray(x.astype(np.float16 if MMDT == F16 else np.float32))

    def pad_din(WT):
        # x-embedding partition layout: word dims at rows 0..49, biword at
        # 64..113 (engine start-partition must be 32-aligned); zero rows
        # contribute nothing to the contraction.
        out = np.zeros((128, WT.shape[1]), WT.dtype)
        out[0:DG] = WT[0:DG]
        out[64:64 + DG] = WT[DG:2 * DG]
        return out

    def reorder_ogi(WT):
        # char gate blocks reordered to (o, g, i) so that [t_i | t_alpha]
        # is contiguous in the XX tile (one Exp covers w_char and w_alpha)
        return np.concatenate([WT[:, H:2 * H], WT[:, 2 * H:3 * H], WT[:, 0:H]], axis=1)

    sh = {}
    sh["WihT"] = mm(pad_din(reorder_ogi(gate_scale(W_ih.T, (0.5, 0.5, 1.0)))))
    sh["WhhT"] = mm(reorder_ogi(gate_scale(W_hh.T, (0.25, 0.25, 0.5))))
    sh["WwihT"] = mm(gate_scale(Ww_ih.T, (0.5, 0.5, 1.0)))      # [50,384]
    sh["WwhhT"] = mm(gate_scale(Ww_hh.T, (0.25, 0.25, 0.5)))    # [128,384]
    sh["WaihT"] = mm(pad_din(0.5 * Wa_ih.T))                     # [128,128]
    sh["WahhT"] = mm(0.25 * Wa_hh.T)                             # [128,128]
    sh["WtagT"] = np.ascontiguousarray(
        0.5 * (W_tag[:, :H] + W_tag[:, H:]).T.astype(np.float32))  # [128,20]
    bl = np.stack([0.5 * b_l[H:2 * H], b_l[2 * H:3 * H], 0.5 * b_l[0:H]], axis=1)  # (o,g,i)
    bw = np.stack([0.5 * b_w[0:H], 0.5 * b_w[H:2 * H], b_w[2 * H:3 * H]], axis=1)
    sh["blstm3"] = np.ascontiguousarray(bl, np.float32)          # [128,3]
    sh["bword3"] = np.ascontiguousarray(bw, np.float32)          # [128,3]
    sh["balpha"] = np.ascontiguousarray(0.5 * b_a[:, None], np.float32)  # [128,1]
    sh["btag"] = np.ascontiguousarray(
        np.broadcast_to(b_tag[None, :], (H, NL)), np.float32)    # [128,20]
    sh["iotmb"] = np.ascontiguousarray(
        np.broadcast_to(np.arange(NL, dtype=np.float32)[None, :] - 1e4, (H, NL)))
    sh["word_table"] = f(inputs["word_table"])
    sh["biword_table"] = f(inputs["biword_table"])
    sh["gaz_table"] = f(inputs["gaz_table"])
    return sh


def build_nc(steps, NA, NB, t_run=T):
    """Emit the SPMD program (same for all cores)."""
    NAp = max(128, ((NA + 127) // 128) * 128)
    n_max = max([sd["n"] for sd in steps] + [1])
    nT4 = t_run // 128 if t_run % 128 == 0 else None
    assert t_run % 128 == 0 or t_run < 128

    nc = bass.Bass()
    dp = nc.declare_dram_parameter
    wtab = dp("word_table", [V_WORD, D_WORD], F32, isOutput=False)
    btab = dp("biword_table", [V_BIWORD, D_BIWORD], F32, isOutput=False)
    gtab = dp("gaz_table", [V_GAZ, DG], F32, isOutput=False)
    wid = dp("wid", [t_run], I32, isOutput=False)
    bid = dp("bid", [t_run], I32, isOutput=False)
    gid = dp("gid", [NAp], I32, isOutput=False)
    WihT = dp("WihT", [128, 3 * H], MMDT, isOutput=False)
    WhhT = dp("WhhT", [H, 3 * H], MMDT, isOutput=False)
    WwihT = dp("WwihT", [DG, 3 * H], MMDT, isOutput=False)
    WwhhT = dp("WwhhT", [H, 3 * H], MMDT, isOutput=False)
    WaihT = dp("WaihT", [128, H], MMDT, isOutput=False)
    WahhT = dp("WahhT", [H, H], MMDT, isOutput=False)
    WtagT = dp("WtagT", [H, NL], F32, isOutput=False)
    blstm3 = dp("blstm3", [H, 3], F32, isOutput=False)
    bword3 = dp("bword3", [H, 3], F32, isOutput=False)
    balpha = dp("balpha", [H, 1], F32, isOutput=False)
    btag = dp("btag", [H, NL], F32, isOutput=False)
    iotmb = dp("iotmb", [H, NL], F32, isOutput=False)
    maskf = dp("maskf", [H, max(NA, 1)], F32, isOutput=False)
    hwm = dp("hwm", [H, max(NB, 1)], F32, isOutput=False)
    maskT = dp("maskT", [H, max(1, (t_run + 127) // 128)], F32, isOutput=False)
    tags = dp("tags", [t_run], I32, isOutput=True)

    with TileContext(nc) as tc:
        with tc.tile_pool(name="const", bufs=1) as cp:
            # persistent tiles
            Hh = cp.tile([H, t_run], MMDT)   # h2 history (fp16, matmul-ready)
            nc.gpsimd.memset(Hh[:], 0.0)
            Cc = cp.tile([H, t_run], F32)     # c history
            nc.gpsimd.memset(Cc[:], 0.0)
            Hf = cp.tile([H, t_run], F32)     # h2 history (f32, for tag head)
            xpret = cp.tile([H, 3 * t_run], F32)  # interleaved: col 3*j+g
            apre = cp.tile([H, t_run], F32)
            wgpre3 = cp.tile([H, 3 * max(NA, 1)], MMDT)
            wgpre3lo = cp.tile([H, 3 * max(NA, 1)], MMDT)
            mft = cp.tile([H, max(NA, 1)], F32)
            nc.sync.dma_start(out=mft[:], in_=maskf[:])
            hwt = cp.tile([H, max(NB, 1)], F32)
            nc.sync.dma_start(out=hwt[:], in_=hwm[:])
            mTt = cp.tile([H, max(1, (t_run + 127) // 128)], F32)
            nc.sync.dma_start(out=mTt[:], in_=maskT[:])
            half = cp.tile([H, 1], F32)
            nc.gpsimd.memset(half[:], 0.5)
            wih = cp.tile([128, 3 * H], MMDT)
            nc.sync.dma_start(out=wih[:], in_=WihT[:])
            whh = cp.tile([H, 3 * H], MMDT)
            nc.sync.dma_start(out=whh[:], in_=WhhT[:])
            wwih = cp.tile([DG, 3 * H], MMDT)
            nc.sync.dma_start(out=wwih[:], in_=WwihT[:])
            wwhh = cp.tile([H, 3 * H], MMDT)
            nc.sync.dma_start(out=wwhh[:], in_=WwhhT[:])
            waih = cp.tile([128, H], MMDT)
            nc.sync.dma_start(out=waih[:], in_=WaihT[:])
            wahh = cp.tile([H, H], MMDT)
            nc.sync.dma_start(out=wahh[:], in_=WahhT[:])
            wtag = cp.tile([H, NL], F32)
            nc.sync.dma_start(out=wtag[:], in_=WtagT[:])
            bl3 = cp.tile([H, 3], F32)
            nc.sync.dma_start(out=bl3[:], in_=blstm3[:])
            bw3 = cp.tile([H, 3], F32)
            nc.sync.dma_start(out=bw3[:], in_=bword3[:])
            bal = cp.tile([H, 1], F32)
            nc.sync.dma_start(out=bal[:], in_=balpha[:])
            btg = cp.tile([H, NL], F32)
            nc.sync.dma_start(out=btg[:], in_=btag[:])
            iot = cp.tile([H, NL], F32)
            nc.sync.dma_start(out=iot[:], in_=iotmb[:])
            ident = cp.tile([128, 128], F32)
            make_identity(nc, ident[:])
            ident16 = cp.tile([128, 128], MMDT)
            nc.vector.tensor_copy(out=ident16[:], in_=ident[:])

            xT16 = cp.tile([128, t_run], MMDT)
            nc.gpsimd.memset(xT16[:], 0.0)
            geT16 = cp.tile([DG, NAp], MMDT)

            # ---------------- pre-stage ----------------
            with tc.tile_pool(name="prew", bufs=2) as pw, \
                 tc.tile_pool(name="prep", bufs=2, space="PSUM") as pp, \
                 tc.tile_pool(name="prep512", bufs=2, space="PSUM") as pp5:
                nch = (t_run + 127) // 128

                def gather(tbl, idx_dram, n_rows, dst16, dst_row0):
                    nchunks = (n_rows + 127) // 128
                    for c in range(nchunks):
                        lo = c * 128
                        nr = min(128, n_rows - lo)
                        it = pw.tile([128, 1], I32, tag="idx")
                        nc.sync.dma_start(out=it[:nr], in_=idx_dram[lo:lo + nr, None])
                        emb = pw.tile([128, DG], F32, tag="emb")
                        nc.gpsimd.indirect_dma_start(
                            out=emb[:nr], out_offset=None, in_=tbl[:],
                            in_offset=bass.IndirectOffsetOnAxis(ap=it[:nr, :1], axis=0))
                        tp = pp.tile([DG, 128], F32, tag="tp", space="PSUM")
                        nc.tensor.transpose(out=tp[:, :nr], in_=emb[:nr], identity=ident[:nr, :nr])
                        nc.scalar.activation(
                            out=dst16[dst_row0:dst_row0 + DG, lo:lo + nr],
                            in_=tp[:, :nr], func=AF.Identity)

                gather(wtab, wid, t_run, xT16, 0)
                gather(btab, bid, t_run, xT16, 64)
                gather(gtab, gid, NAp, geT16, 0)

                # xpre3 / apre
                for g in range(3):
                    done = 0
                    while done < t_run:
                        n_ = min(512, t_run - done)
                        ps = pp5.tile([H, 512], F32, tag="ps", space="PSUM")
                        nc.tensor.matmul(out=ps[:, :n_], lhsT=wih[:, g * H:(g + 1) * H],
                                         rhs=xT16[:, done:done + n_], start=True, stop=True)
                        nc.scalar.activation(
                            out=xpret[:].rearrange("p (t g) -> p t g", g=3)[:, done:done + n_, g],
                            in_=ps[:, :n_], func=AF.Identity, bias=bl3[:, g:g + 1])
                        done += n_
                done = 0
                while done < t_run:
                    n_ = min(512, t_run - done)
                    ps = pp5.tile([H, 512], F32, tag="ps", space="PSUM")
                    nc.tensor.matmul(out=ps[:, :n_], lhsT=waih[:],
                                     rhs=xT16[:, done:done + n_], start=True, stop=True)
                    nc.scalar.activation(out=apre[:, done:done + n_], in_=ps[:, :n_],
                                         func=AF.Identity, bias=bal[:, 0:1])
                    done += n_
                if NA > 0:
                    for g in range(3):
                        done = 0
                        while done < NA:
                            n_ = min(512, NA - done)
                            ps = pp5.tile([H, 512], F32, tag="ps", space="PSUM")
                            nc.tensor.matmul(out=ps[:, :n_], lhsT=wwih[:, g * H:(g + 1) * H],
                                             rhs=geT16[:, done:done + n_], start=True, stop=True)
                            w32 = pw.tile([H, 512], F32, tag="w32")
                            nc.scalar.activation(
                                out=w32[:, :n_],
                                in_=ps[:, :n_], func=AF.Identity, bias=bw3[:, g:g + 1])
                            sl = slice(g * NA + done, g * NA + done + n_)
                            nc.vector.tensor_copy(out=wgpre3[:, sl], in_=w32[:, :n_])
                            lo32 = pw.tile([H, 512], F32, tag="lo32")
                            nc.vector.tensor_tensor(out=lo32[:, :n_], in0=w32[:, :n_],
                                                    in1=wgpre3[:, sl], op=ALU.subtract)
                            nc.vector.tensor_copy(out=wgpre3lo[:, sl], in_=lo32[:, :n_])
                            done += n_

            # ---------------- scan ----------------
            with tc.tile_pool(name="work", bufs=3) as wk, \
                 tc.tile_pool(name="spsum", bufs=2, space="PSUM") as sp:
                wg3v = wgpre3[:].rearrange("p (g t) -> p g t", g=3)
                wg3lv = wgpre3lo[:].rearrange("p (g t) -> p g t", g=3)
                for sd in steps:
                    j = sd["j"]
                    if j == 0:
                        # all cores coupled at j=0: c0 = sig(i)*g, h = sig(o)*tanh(c0)
                        th0 = wk.tile([H, 3], F32, tag="XX")
                        nc.scalar.activation(out=th0[:], in_=xpret[:, 0:3], func=AF.Tanh)
                        c2 = wk.tile([H, 1], F32, tag="c2")
                        nc.vector.scalar_tensor_tensor(
                            out=c2[:], in0=th0[:, 2:3], scalar=1.0, in1=th0[:, 1:2],
                            op0=ALU.add, op1=ALU.mult)
                        nc.vector.tensor_scalar(
                            out=Cc[:, 0:1], in0=c2[:], scalar1=0.5, scalar2=None,
                            op0=ALU.mult)
                        tcn = wk.tile([H, 1], F32, tag="tc")
                        nc.scalar.activation(out=tcn[:], in_=Cc[:, 0:1], func=AF.Tanh)
                        nc.vector.scalar_tensor_tensor(
                            out=Hh[:, 0:1], in0=th0[:, 0:1], scalar=1.0, in1=tcn[:],
                            op0=ALU.add, op1=ALU.mult)
                        nc.vector.scalar_tensor_tensor(
                            out=Hf[:, 0:1], in0=th0[:, 0:1], scalar=1.0, in1=tcn[:],
                            op0=ALU.add, op1=ALU.mult)
                        continue

                    nb, n, off = sd["nb"], sd["n"], sd["off"]
                    ws = sd["wordstep"]
                    c_prev = Cc[:, j - 1:j]
                    rhs_h = Hh[:, j - 1:j]
                    nr = len(sd["rep_lags"]) if ws else 0

                    # char gates (o,g,i): psum + DVE preadd + tanh (off-spine)
                    pa = sp.tile([H, 3], F32, tag="pa", space="PSUM")
                    for g in range(3):
                        nc.tensor.matmul(out=pa[:, g:g + 1], lhsT=whh[:, g * H:(g + 1) * H],
                                         rhs=rhs_h, start=True, stop=True)
                    ctt = wk.tile([H, 3], F32, tag="ctt")
                    nc.vector.tensor_tensor(out=ctt[:], in0=pa[:, 0:3],
                                            in1=xpret[:, 3 * j:3 * j + 3], op=ALU.add)
                    xx = wk.tile([H, 3 + n_max], F32, tag="XX")
                    # T2: t_o, t_g, t_i at xx[:,0:3]

                    if ws:
                        crep = None
                        if nr:
                            s16 = wk.tile([H, n_max], MMDT, tag="s16")
                            nc.vector.tensor_copy(out=s16[:, 0:nb], in_=Hh[:, j - nb:j])
                            crep = wk.tile([H, max(nr, 1)], F32, tag="crep")
                            for r, dd in enumerate(sd["rep_lags"]):
                                nc.vector.tensor_copy(out=s16[:, nb + r:nb + r + 1],
                                                      in_=Hh[:, j - dd:j - dd + 1])
                                nc.vector.tensor_copy(out=crep[:, r:r + 1],
                                                      in_=Cc[:, j - dd:j - dd + 1])
                            rhs_all = s16[:, 0:n]
                        else:
                            rhs_all = Hh[:, j - nb:j]

                        # word gates: psum preloaded with wgpre (identity matmul,
                        # no h dependence -> runs early), then 3 gate matmuls
                        pwg = sp.tile([H, 3 * n_max], F32, tag="pw", space="PSUM")
                        nc.tensor.matmul(out=pwg[:, 0:3 * n].rearrange("p (g n) -> p g n", g=3),
                                         lhsT=ident16[:], rhs=wg3v[:, :, off:off + n],
                                         start=True, stop=False)
                        nc.tensor.matmul(out=pwg[:, 0:3 * n].rearrange("p (g n) -> p g n", g=3),
                                         lhsT=ident16[:], rhs=wg3lv[:, :, off:off + n],
                                         start=False, stop=False)
                        for g in range(3):
                            nc.tensor.matmul(out=pwg[:, g * n:(g + 1) * n],
                                             lhsT=wwhh[:, g * H:(g + 1) * H],
                                             rhs=rhs_all, start=False, stop=(g == 2))
                        tw = wk.tile([H, 3 * n_max], F32, tag="TW")
                        nc.scalar.activation(out=tw[:, 0:3 * n], in_=pwg[:, 0:3 * n],
                                             func=AF.Tanh)
                        # m1 = (t_iw+1)*t_gw, m2 = (t_fw+1)*c_s  (fp16, 2x scaled)
                        m1 = wk.tile([H, n_max], MMDT, tag="m1")
                        nc.vector.scalar_tensor_tensor(
                            out=m1[:, 0:n], in0=tw[:, 0:n], scalar=1.0,
                            in1=tw[:, 2 * n:3 * n], op0=ALU.add, op1=ALU.mult)
                        m2 = wk.tile([H, n_max], MMDT, tag="m2")
                        nc.vector.scalar_tensor_tensor(
                            out=m2[:, 0:nb], in0=tw[:, n:n + nb], scalar=1.0,
                            in1=Cc[:, j - nb:j], op0=ALU.add, op1=ALU.mult)
                        if nr:
                            nc.vector.scalar_tensor_tensor(
                                out=m2[:, nb:n], in0=tw[:, n + nb:n + n], scalar=1.0,
                                in1=crep[:, 0:nr], op0=ALU.add, op1=ALU.mult)
                        # alpha psum: 0.25*Wa.T @ (m1 + m2) via accumulation
                        pal = sp.tile([H, n_max], F32, tag="pal", space="PSUM")
                        nc.tensor.matmul(out=pal[:, 0:n], lhsT=wahh[:],
                                         rhs=m1[:, 0:n], start=True, stop=False)
                        nc.tensor.matmul(out=pal[:, 0:n], lhsT=wahh[:],
                                         rhs=m2[:, 0:n], start=False, stop=True)
                        nc.scalar.activation(out=xx[:, 0:3], in_=ctt[:], func=AF.Tanh)
                        nc.scalar.activation(out=xx[:, 3:3 + n], in_=pal[:, 0:n],
                                             func=AF.Tanh, bias=apre[:, j:j + 1])
                        ee = wk.tile([H, 1 + n_max], F32, tag="ee")
                        nc.scalar.activation(out=ee[:, 0:1 + n], in_=xx[:, 2:3 + n],
                                             func=AF.Exp, scale=0.5, bias=half[:, 0:1])
                        # off-spine: cw' = m12a+m12b ; mcw = 0.5*mask*cw'
                        cwf = wk.tile([H, n_max], F32, tag="cwf")
                        nc.vector.tensor_tensor(out=cwf[:, 0:n], in0=m1[:, 0:n],
                                                in1=m2[:, 0:n], op=ALU.add)
                        mcw = wk.tile([H, n_max], F32, tag="mcw")
                        nc.vector.scalar_tensor_tensor(
                            out=mcw[:, 0:n], in0=mft[:, off:off + n], scalar=0.5,
                            in1=cwf[:, 0:n], op0=ALU.mult, op1=ALU.mult)
                        wm = wk.tile([H, n_max], F32, tag="wm")
                        s0 = wk.tile([H, 1], F32, tag="s0")
                        nc.vector.scalar_tensor_tensor(
                            out=wm[:, 0:n], in0=ee[:, 1:1 + n], scalar=1.0,
                            in1=mft[:, off:off + n], op0=ALU.bypass, op1=ALU.mult,
                            accum_out=s0[:])
                        wcw = wk.tile([H, n_max], F32, tag="wcw")
                        s1 = wk.tile([H, 1], F32, tag="s1")
                        nc.vector.scalar_tensor_tensor(
                            out=wcw[:, 0:n], in0=ee[:, 1:1 + n], scalar=1.0,
                            in1=mcw[:, 0:n], op0=ALU.bypass, op1=ALU.mult,
                            accum_out=s1[:])
                        den = wk.tile([H, 1], F32, tag="den")
                        nc.scalar.activation(out=den[:], in_=s0[:], func=AF.Identity,
                                             bias=ee[:, 0:1])
                        rcp = wk.tile([H, 1], F32, tag="rcp")
                        nc.vector.reciprocal(out=rcp[:], in_=den[:])
                        num = wk.tile([H, 1], F32, tag="num")
                        nc.vector.scalar_tensor_tensor(
                            out=num[:], in0=xx[:, 1:2], scalar=ee[:, 0:1], in1=s1[:],
                            op0=ALU.mult, op1=ALU.add)
                        tcn = wk.tile([H, 1], F32, tag="tc")
                        if sd["blend"]:
                            csoft = wk.tile([H, 1], F32, tag="csoft")
                            nc.vector.tensor_tensor(out=csoft[:], in0=num[:],
                                                    in1=rcp[:], op=ALU.mult)
                            dd_ = wk.tile([H, 1], F32, tag="dd")
                            nc.vector.tensor_tensor(out=dd_[:], in0=xx[:, 1:2],
                                                    in1=c_prev, op=ALU.subtract)
                            e2 = wk.tile([H, 1], F32, tag="e2")
                            nc.vector.scalar_tensor_tensor(
                                out=e2[:], in0=xx[:, 2:3], scalar=1.0, in1=dd_[:],
                                op0=ALU.add, op1=ALU.mult)
                            ccpl = wk.tile([H, 1], F32, tag="ccpl")
                            nc.vector.scalar_tensor_tensor(
                                out=ccpl[:], in0=e2[:], scalar=0.5, in1=c_prev,
                                op0=ALU.mult, op1=ALU.add)
                            dif = wk.tile([H, 1], F32, tag="dif")
                            nc.vector.tensor_tensor(out=dif[:], in0=csoft[:],
                                                    in1=ccpl[:], op=ALU.subtract)
                            hwc = sd["hw_col"]
                            nc.vector.scalar_tensor_tensor(
                                out=Cc[:, j:j + 1], in0=dif[:],
                                scalar=hwt[:, hwc:hwc + 1], in1=ccpl[:],
                                op0=ALU.mult, op1=ALU.add)
                            nc.scalar.activation(out=tcn[:], in_=Cc[:, j:j + 1],
                                                 func=AF.Tanh)
                        else:
                            # spine: tanh(num/den) via per-partition scale; the
                            # Cc history write happens off-spine in parallel
                            nc.scalar.activation(out=tcn[:], in_=num[:],
                                                 func=AF.Tanh, scale=rcp[:, 0:1])
                            nc.vector.tensor_tensor(out=Cc[:, j:j + 1], in0=num[:],
                                                    in1=rcp[:], op=ALU.mult)
                    else:
                        # coupled path only
                        nc.scalar.activation(out=xx[:, 0:3], in_=ctt[:], func=AF.Tanh)
                        dd_ = wk.tile([H, 1], F32, tag="dd")
                        nc.vector.tensor_tensor(out=dd_[:], in0=xx[:, 1:2],
                                                in1=c_prev, op=ALU.subtract)
                        e2 = wk.tile([H, 1], F32, tag="e2")
                        nc.vector.scalar_tensor_tensor(
                            out=e2[:], in0=xx[:, 2:3], scalar=1.0, in1=dd_[:],
                            op0=ALU.add, op1=ALU.mult)
                        nc.vector.scalar_tensor_tensor(
                            out=Cc[:, j:j + 1], in0=e2[:], scalar=0.5, in1=c_prev,
                            op0=ALU.mult, op1=ALU.add)
                        tcn = wk.tile([H, 1], F32, tag="tc")
                        nc.scalar.activation(out=tcn[:], in_=Cc[:, j:j + 1],
                                             func=AF.Tanh)

                    nc.vector.scalar_tensor_tensor(
                        out=Hh[:, j:j + 1], in0=xx[:, 0:1], scalar=1.0, in1=tcn[:],
                        op0=ALU.add, op1=ALU.mult)
                    nc.vector.scalar_tensor_tensor(
                        out=Hf[:, j:j + 1], in0=xx[:, 0:1], scalar=1.0, in1=tcn[:],
                        op0=ALU.add, op1=ALU.mult)

                # ---------------- epilogue: tag head + argmax ----------------
                nchunks = (t_run + 127) // 128
                for c in range(nchunks):
                    lo = c * 128
                    nr = min(128, t_run - lo)
                    pt = sp.tile([128, NL], F32, tag="pt", space="PSUM")
                    nc.tensor.matmul(out=pt[:nr], lhsT=Hf[:, lo:lo + nr],
                                     rhs=wtag[:], start=True, stop=True)
                    lg = wk.tile([128, NL], F32, tag="lg")
                    nc.vector.tensor_tensor(out=lg[:nr], in0=pt[:nr], in1=btg[:nr],
                                            op=ALU.add)
                    mx = wk.tile([128, 1], F32, tag="mx")
                    nc.vector.tensor_reduce(out=mx[:nr], in_=lg[:nr], axis=AX.X,
                                            op=ALU.max)
                    eq = wk.tile([128, NL], F32, tag="eq")
                    nc.vector.tensor_scalar(out=eq[:nr], in0=lg[:nr],
                                            scalar1=mx[:nr, 0:1], scalar2=None,
                                            op0=ALU.is_equal)
                    j2 = wk.tile([128, NL], F32, tag="j2")
                    im = wk.tile([128, 1], F32, tag="im")
                    nc.vector.tensor_tensor(out=j2[:nr], in0=eq[:nr], in1=iot[:nr],
                                            op=ALU.mult)
                    nc.vector.tensor_reduce(out=im[:nr], in_=j2[:nr], axis=AX.X,
                                            op=ALU.min)
                    tf = wk.tile([128, 1], F32, tag="tf")
                    nc.vector.scalar_tensor_tensor(
                        out=tf[:nr], in0=im[:nr], scalar=1e4, in1=mTt[:nr, c:c + 1],
                        op0=ALU.add, op1=ALU.mult)
                    ti = wk.tile([128, 1], I32, tag="ti")
                    nc.vector.tensor_copy(out=ti[:nr], in_=tf[:nr])
                    nc.sync.dma_start(out=tags[lo:lo + nr, None], in_=ti[:nr])

    return nc


def make_in_maps(inputs, steps, NA, NB, t_run=T):
    sh = prep_shared(inputs, t_run)
    NAp = max(128, ((NA + 127) // 128) * 128)
    in_maps = []
    mask_in = np.asarray(inputs["mask"])
    for b in range(B):
        gid, msk, hw = pack_core(b, steps, inputs["gaz_word_ids"],
                                 inputs["gaz_starts"], inputs["gaz_mask"],
                                 NA, NB, t_run)
        gidp = np.zeros(NAp, np.int32)
        gidp[:NA] = gid
        nch = max(1, (t_run + 127) // 128)
        mT = np.zeros((H, nch), np.float32)
        mrow = mask_in[b, :t_run].astype(np.float32)
        for c in range((t_run + 127) // 128):
            nr = min(128, t_run - c * 128)
            mT[:nr, c] = mrow[c * 128:c * 128 + nr]
        m = dict(sh)
        m["wid"] = np.asarray(inputs["word_inputs"])[b, :t_run].astype(np.int32).copy()
        m["bid"] = np.asarray(inputs["biword_inputs"])[b, :t_run].astype(np.int32).copy()
        m["gid"] = gidp
        m["maskf"] = np.ascontiguousarray(
            np.broadcast_to(msk[None, :], (H, max(NA, 1)))) if NA > 0 else np.zeros((H, 1), np.float32)
        m["hwm"] = np.ascontiguousarray(
            np.broadcast_to(hw[None, :], (H, max(NB, 1))))
        m["maskT"] = mT
        in_maps.append(m)
    return in_maps


def kernel(**inputs) -> np.ndarray:
    steps, NA, NB = build_structure(inputs["gaz_starts"], inputs["gaz_mask"], T)
    nc = build_nc(steps, NA, NB, T)
    _legalize_single_wait(nc)
    in_maps = make_in_maps(inputs, steps, NA, NB, T)
    res = run_bass_kernel_spmd(nc, in_maps, list(range(B)))
    out = np.stack([res.results[b]["tags"] for b in range(B)], axis=0)
    return out.astype(np.int32)



# revision 12
# speedup vs baseline: 1.2963x; 1.2963x over previous
"""Lattice-LSTM NER tagger (nn_BiLSTM_88484916232709) on 8 TRN2 NeuronCores.

Strategy: data-parallel over the batch (B=8 -> one row per core), SPMD.
The lattice scan is sequential in T; edges ending at step j have lag
d = len-1 in [1, 8].  v5 design: a 2-stage software pipeline per step --
all lag>=2 edge work for step j+1 (word gates, alpha, exp weights,
partial softmax sums) is computed during step j (it only needs h/c up to
j-1), leaving a minimal critical chain per step: lag-1 word gates (own
psum bank, tanh fires without waiting for char), char gates (second
bank), the lag-1 alpha/exp path, a seed-folded masked accumulate into
den/num (early partial sums live in columns adjacent to the late exp
values, so one scalar_tensor_tensor+accum produces den/num directly),
reciprocal, tanh, h-write.

Duplicate-lag edges ("replicas") are packed into padded contiguous
lag-runs ("blocks") so every word-gate matmul takes a contiguous Hh/Cc
window as rhs -- no per-column staging copies.  The embedding-gather /
pregate pre-stage is chunked (128 cols) and interleaved into the scan
(emitted on a just-in-time cursor), hiding ~200us behind the recurrence.
All pregates are fp16 (validated: <=1 tag flip vs golden).

Sigmoids are tanh-folded: sig(x) = 0.5*tanh(0.5x)+0.5 with scales folded
into the prestage weights, so one ACT table set (tanh/exp/identity)
serves the whole kernel.
"""

import numpy as np

import concourse.bass as bass
import concourse.mybir as mybir
from concourse.tile import TileContext
from concourse.bass_utils import run_bass_kernel_spmd
from concourse.masks import make_identity

B, T, K, H = 8, 512, 8, 128
DIN, DG, NL = 100, 50, 20
V_WORD, V_BIWORD, V_GAZ = 100000, 200000, 300000
D_WORD, D_BIWORD = 50, 50

F32 = mybir.dt.float32
F16 = mybir.dt.float16
I32 = mybir.dt.int32
AF = mybir.ActivationFunctionType
ALU = mybir.AluOpType
AX = mybir.AxisListType

MMDT = F16
ZBIG = 1.0e6


def _legalize_single_wait(nc):
    """This walrus build allows at most one sync-wait per instruction.
    Peel extra waits onto same-engine single-wait EventSemaphore insts."""
    k = 0
    for f in nc.m.functions:
        for bb in f.blocks:
            insts = bb.instructions
            i = 0
            while i < len(insts):
                inst = insts[i]
                si = getattr(inst, "sync_info", None)
                if si is not None and len(si.on_wait) > 1:
                    extra = list(si.on_wait[:-1])
                    keep = si.on_wait[-1]
                    peeled = []
                    for w in extra:
                        ev = mybir.InstEventSemaphore(
                            name=f"sw{k}", ins=[], outs=[]
                        )
                        k += 1
                        ev.engine = inst.engine
                        ev.sync_info = mybir.SyncInfo(on_wait=[w], on_update=[])
                        peeled.append(ev)
                    si.on_wait.clear()
                    si.on_wait.append(keep)
                    insts[i:i] = peeled
                    i += len(peeled)
                i += 1
    return k


def build_structure(gaz_starts, gaz_mask, t_run):
    """Core-uniform per-step schedule (early/late split, lag blocks)."""
    gs = np.asarray(gaz_starts)
    gm = np.asarray(gaz_mask).astype(bool)
    lag = np.arange(t_run)[None, :, None] - gs[:, :t_run]
    steps = []
    off = 0
    nb_blend = 0
    for j in range(t_run):
        nb = min(8, j)
        counts = np.zeros((B, nb + 1), np.int64)
        for b in range(B):
            for k in range(K):
                if gm[b, j, k]:
                    dd = int(lag[b, j, k])
                    assert 1 <= dd <= nb
                    counts[b, dd] += 1
        cap = counts.max(axis=0) if nb > 0 else np.zeros(1, np.int64)
        total = int(counts.sum())
        wordstep = total > 0
        per_core_any = counts.sum(axis=1) > 0
        blend = wordstep and not per_core_any.all()
        r = int(cap[1]) if nb >= 1 else 0
        eblocks = []  # (d_hi, width, dup)
        if wordstep and nb >= 2:
            eblocks.append((nb, nb - 1, 0))
            rem = []
            for d in range(2, nb + 1):
                rem += [d] * max(0, int(cap[d]) - 1)
            dupi = 1
            while rem:
                seen = sorted(set(rem), reverse=True)
                d1, d2 = seen[0], seen[-1]
                eblocks.append((d1, d1 - d2 + 1, dupi))
                used = set()
                nrem = []
                for d in rem:
                    if d in used:
                        nrem.append(d)
                    else:
                        used.add(d)
                rem = nrem
                dupi += 1
        ne = sum(w for _, w, _ in eblocks)
        sd = dict(
            j=j, nb=nb, wordstep=wordstep, blend=blend,
            hw_col=nb_blend if blend else None,
            r=r, ne=ne, eblocks=eblocks, off=off,
        )
        if wordstep:
            off += ne + r
        if blend:
            nb_blend += 1
        steps.append(sd)
    return steps, off, nb_blend


def step_cols(sd):
    c = 0
    for d_hi, w, dup in sd["eblocks"]:
        for i in range(w):
            yield c, d_hi - i, dup
            c += 1
    for i in range(sd["r"]):
        yield c, 1, i
        c += 1


def pack_core(b, steps, gaz_word_ids, gaz_starts, gaz_mask, NA, NB, t_run):
    """Per-core data: gaz ids, masks (mft), half-masks (hm), den-fold in1
    (mld: [1, 0, 1, mask-late...] per wordstep), blend z columns."""
    gid = np.zeros(max(NA, 1), np.int32)
    mft = np.zeros(max(NA, 1), np.float16)
    hm = np.zeros(max(NA, 1), np.float16)
    LM3 = sum((3 + sd["r"]) for sd in steps if sd["wordstep"])
    mld = np.zeros(max(LM3, 1), np.float16)
    zc = np.full(max(NB, 1), ZBIG, np.float32)
    gids = np.asarray(gaz_word_ids)
    gs = np.asarray(gaz_starts)
    gm = np.asarray(gaz_mask).astype(bool)
    lmoff = 0
    for sd in steps:
        j = sd["j"]
        if not sd["wordstep"]:
            continue
        by_lag = {}
        for k in range(K):
            if gm[b, j, k]:
                dd = j - int(gs[b, j, k])
                by_lag.setdefault(dd, []).append(int(gids[b, j, k]))
        off = sd["off"]
        mld[lmoff] = 1.0      # s0e passthrough
        mld[lmoff + 1] = 0.0  # kill s1e column in the den fold
        mld[lmoff + 2] = 1.0  # w_char
        for c, d, dup in step_cols(sd):
            lst = by_lag.get(d, [])
            if len(lst) > dup:
                gid[off + c] = lst[dup]
                mft[off + c] = 1.0
                hm[off + c] = 0.5
                if d == 1:
                    mld[lmoff + 3 + (c - sd["ne"])] = 1.0
        if sd["hw_col"] is not None:
            zc[sd["hw_col"]] = 0.0 if by_lag else ZBIG
        lmoff += 3 + sd["r"]
    return gid, mft, hm, mld, zc, LM3


def prep_shared(inputs, t_run=T):
    f = lambda x: np.ascontiguousarray(np.asarray(x, np.float32))
    W_ih, W_hh, b_l = f(inputs["W_ih"]), f(inputs["W_hh"]), f(inputs["b_lstm"])
    Wa_ih, Wa_hh, b_a = f(inputs["Wa_ih"]), f(inputs["Wa_hh"]), f(inputs["b_alpha"])
    Ww_ih, Ww_hh, b_w = f(inputs["Ww_ih"]), f(inputs["Ww_hh"]), f(inputs["b_word"])
    W_tag, b_tag = f(inputs["W_tag"]), f(inputs["b_tag"])

    def gate_scale(WT, scales):
        out = WT.copy()
        for g, s in enumerate(scales):
            out[:, g * H:(g + 1) * H] *= s
        return out

    mm = lambda x: np.ascontiguousarray(x.astype(np.float16))

    def pad_din(WT):
        out = np.zeros((128, WT.shape[1]), WT.dtype)
        out[0:DG] = WT[0:DG]
        out[64:64 + DG] = WT[DG:2 * DG]
        return out

    def reorder_ogi(WT):
        return np.concatenate([WT[:, H:2 * H], WT[:, 2 * H:3 * H], WT[:, 0:H]], axis=1)

    sh = {}
    sh["WihT"] = mm(pad_din(reorder_ogi(gate_scale(W_ih.T, (0.5, 0.5, 1.0)))))
    sh["WhhT"] = mm(reorder_ogi(gate_scale(W_hh.T, (0.25, 0.25, 0.5))))
    sh["WwihT"] = mm(gate_scale(Ww_ih.T, (0.5, 0.5, 1.0)))
    sh["WwhhT"] = mm(gate_scale(Ww_hh.T, (0.25, 0.25, 0.5)))
    sh["WaihT"] = mm(pad_din(0.5 * Wa_ih.T))
    sh["WahhT"] = mm(0.25 * Wa_hh.T)
    sh["WtagT"] = mm(0.5 * (W_tag[:, :H] + W_tag[:, H:]).T)
    bl = np.stack([0.5 * b_l[H:2 * H], b_l[2 * H:3 * H], 0.5 * b_l[0:H]], axis=1)
    bw = np.stack([0.5 * b_w[0:H], 0.5 * b_w[H:2 * H], b_w[2 * H:3 * H]], axis=1)
    sh["blstm3"] = np.ascontiguousarray(bl, np.float32)
    sh["bword3"] = np.ascontiguousarray(bw, np.float32)
    sh["balpha"] = np.ascontiguousarray(0.5 * b_a[:, None], np.float32)
    sh["btag"] = np.ascontiguousarray(
        np.broadcast_to(b_tag[None, :], (H, NL)), np.float32)
    sh["iotmb"] = np.ascontiguousarray(
        np.broadcast_to(np.arange(NL, dtype=np.float32)[None, :] - 1e4, (H, NL)))
    sh["word_table"] = f(inputs["word_table"])
    sh["biword_table"] = f(inputs["biword_table"])
    sh["gaz_table"] = f(inputs["gaz_table"])
    return sh


def build_nc(steps, NA, NB, t_run=T):
    NAp = max(128, ((NA + 127) // 128) * 128)
    LM3 = sum((3 + sd["r"]) for sd in steps if sd["wordstep"])
    r_max = max([sd["r"] for sd in steps] + [1])
    ne_max = max([sd["ne"] for sd in steps] + [1])

    nc = bass.Bass()
    dp = nc.declare_dram_parameter
    wtab = dp("word_table", [V_WORD, D_WORD], F32, isOutput=False)
    btab = dp("biword_table", [V_BIWORD, D_BIWORD], F32, isOutput=False)
    gtab = dp("gaz_table", [V_GAZ, DG], F32, isOutput=False)
    wid = dp("wid", [t_run], I32, isOutput=False)
    bid = dp("bid", [t_run], I32, isOutput=False)
    gid = dp("gid", [NAp], I32, isOutput=False)
    WihT = dp("WihT", [128, 3 * H], MMDT, isOutput=False)
    WhhT = dp("WhhT", [H, 3 * H], MMDT, isOutput=False)
    WwihT = dp("WwihT", [DG, 3 * H], MMDT, isOutput=False)
    WwhhT = dp("WwhhT", [H, 3 * H], MMDT, isOutput=False)
    WaihT = dp("WaihT", [128, H], MMDT, isOutput=False)
    WahhT = dp("WahhT", [H, H], MMDT, isOutput=False)
    WtagT = dp("WtagT", [H, NL], MMDT, isOutput=False)
    blstm3 = dp("blstm3", [H, 3], F32, isOutput=False)
    bword3 = dp("bword3", [H, 3], F32, isOutput=False)
    balpha = dp("balpha", [H, 1], F32, isOutput=False)
    btag = dp("btag", [H, NL], F32, isOutput=False)
    iotmb = dp("iotmb", [H, NL], F32, isOutput=False)
    mftD = dp("mft", [H, max(NA, 1)], F16, isOutput=False)
    hmD = dp("hm", [H, max(NA, 1)], F16, isOutput=False)
    mldD = dp("mld", [H, max(LM3, 1)], F16, isOutput=False)
    zcD = dp("zc", [H, max(NB, 1)], F32, isOutput=False)
    maskT = dp("maskT", [H, max(1, (t_run + 127) // 128)], F32, isOutput=False)
    tags = dp("tags", [t_run], I32, isOutput=True)

    # step -> offset into mld
    lm_offs = {}
    _lo = 0
    for sd in steps:
        if sd["wordstep"]:
            lm_offs[sd["j"]] = _lo
            _lo += 3 + sd["r"]

    def off_end(k):
        if k >= t_run:
            return NA
        return steps[k]["off"]

    with TileContext(nc) as tc:
        with tc.tile_pool(name="const", bufs=1) as cp:
            Hh = cp.tile([H, t_run], MMDT)
            nc.gpsimd.memset(Hh[:], 0.0)
            Cc = cp.tile([H, t_run], F32)
            nc.gpsimd.memset(Cc[:], 0.0)
            xpre16 = cp.tile([H, 3 * t_run], MMDT)
            apre = cp.tile([H, t_run], F32)
            wgpre3 = cp.tile([H, 3 * max(NA, 1)], MMDT)
            EEL3 = cp.tile([H, 3 + r_max], F32)   # [s0e | s1e | w_char | w-late]
            nc.gpsimd.memset(EEL3[:], 0.0)
            MCN = cp.tile([H, 2 + r_max], F32)    # [1 | t_g | mcw-late]
            nc.gpsimd.memset(MCN[:], 0.0)
            nc.gpsimd.memset(MCN[:, 0:1], 1.0)
            mft = cp.tile([H, max(NA, 1)], F16)
            nc.sync.dma_start(out=mft[:], in_=mftD[:])
            hm = cp.tile([H, max(NA, 1)], F16)
            nc.sync.dma_start(out=hm[:], in_=hmD[:])
            mld = cp.tile([H, max(LM3, 1)], F16)
            nc.sync.dma_start(out=mld[:], in_=mldD[:])
            zct = cp.tile([H, max(NB, 1)], F32)
            nc.sync.dma_start(out=zct[:], in_=zcD[:])
            mTt = cp.tile([H, max(1, (t_run + 127) // 128)], F32)
            nc.sync.dma_start(out=mTt[:], in_=maskT[:])
            half = cp.tile([H, 1], F32)
            nc.gpsimd.memset(half[:], 0.5)
            wih = cp.tile([128, 3 * H], MMDT)
            nc.sync.dma_start(out=wih[:], in_=WihT[:])
            whh = cp.tile([H, 3 * H], MMDT)
            nc.sync.dma_start(out=whh[:], in_=WhhT[:])
            wwih = cp.tile([DG, 3 * H], MMDT)
            nc.sync.dma_start(out=wwih[:], in_=WwihT[:])
            wwhh = cp.tile([H, 3 * H], MMDT)
            nc.sync.dma_start(out=wwhh[:], in_=WwhhT[:])
            waih = cp.tile([128, H], MMDT)
            nc.sync.dma_start(out=waih[:], in_=WaihT[:])
            wahh = cp.tile([H, H], MMDT)
            nc.sync.dma_start(out=wahh[:], in_=WahhT[:])
            wtag = cp.tile([H, NL], MMDT)
            nc.sync.dma_start(out=wtag[:], in_=WtagT[:])
            bl3 = cp.tile([H, 3], F32)
            nc.sync.dma_start(out=bl3[:], in_=blstm3[:])
            bw3 = cp.tile([H, 3], F32)
            nc.sync.dma_start(out=bw3[:], in_=bword3[:])
            bal = cp.tile([H, 1], F32)
            nc.sync.dma_start(out=bal[:], in_=balpha[:])
            btg = cp.tile([H, NL], F32)
            nc.sync.dma_start(out=btg[:], in_=btag[:])
            iot = cp.tile([H, NL], F32)
            nc.sync.dma_start(out=iot[:], in_=iotmb[:])
            ident = cp.tile([128, 128], F32)
            make_identity(nc, ident[:])
            ident16 = cp.tile([128, 128], MMDT)
            nc.vector.tensor_copy(out=ident16[:], in_=ident[:])

            xT16 = cp.tile([128, t_run], MMDT)
            nc.gpsimd.memset(xT16[:], 0.0)
            geT16 = cp.tile([DG, NAp], MMDT)

            # ---------------- scan (prestage interleaved) ----------------
            with tc.tile_pool(name="wk", bufs=4) as wk, \
                 tc.tile_pool(name="acc", bufs=4) as ak, \
                 tc.tile_pool(name="prew", bufs=2) as pw:
              with tc.tile_pool(name="pl", bufs=2, space="PSUM") as plp, \
                 tc.tile_pool(name="pe", bufs=1, space="PSUM") as pep, \
                 tc.tile_pool(name="pale", bufs=1, space="PSUM") as palp, \
                 tc.tile_pool(name="pc", bufs=2, space="PSUM") as pcp, \
                 tc.tile_pool(name="pp", bufs=2, space="PSUM") as ppp:
                  wg3v = wgpre3[:].rearrange("p (g t) -> p g t", g=3)
                  carry = {}
                  cur = {"x": 0, "gz": 0}

                  def gather_chunk(tbl, idx_dram, lo, nr, dst16, dst_row0):
                      it = pw.tile([128, 1], I32, tag="idx")
                      nc.sync.dma_start(out=it[:nr], in_=idx_dram[lo:lo + nr, None])
                      emb = pw.tile([128, DG], F32, tag="emb")
                      nc.gpsimd.indirect_dma_start(
                          out=emb[:nr], out_offset=None, in_=tbl[:],
                          in_offset=bass.IndirectOffsetOnAxis(ap=it[:nr, :1], axis=0))
                      tp = ppp.tile([128, 128], F32, tag="pp", space="PSUM")
                      nc.tensor.transpose(out=tp[:DG, :nr], in_=emb[:nr],
                                          identity=ident[:nr, :nr])
                      nc.scalar.activation(
                          out=dst16[dst_row0:dst_row0 + DG, lo:lo + nr],
                          in_=tp[:DG, :nr], func=AF.Identity)

                  def emit_x_chunk(c):
                      lo = c * 128
                      nr = min(128, t_run - lo)
                      gather_chunk(wtab, wid, lo, nr, xT16, 0)
                      gather_chunk(btab, bid, lo, nr, xT16, 64)
                      for g in range(3):
                          ps = ppp.tile([128, 128], F32, tag="pp", space="PSUM")
                          nc.tensor.matmul(out=ps[:H, :nr],
                                           lhsT=wih[:, g * H:(g + 1) * H],
                                           rhs=xT16[:, lo:lo + nr],
                                           start=True, stop=True)
                          nc.scalar.activation(
                              out=xpre16[:].rearrange("p (t g) -> p t g", g=3)[:, lo:lo + nr, g],
                              in_=ps[:H, :nr], func=AF.Identity, bias=bl3[:, g:g + 1])
                      ps = ppp.tile([128, 128], F32, tag="pp", space="PSUM")
                      nc.tensor.matmul(out=ps[:H, :nr], lhsT=waih[:],
                                       rhs=xT16[:, lo:lo + nr], start=True, stop=True)
                      nc.scalar.activation(out=apre[:, lo:lo + nr], in_=ps[:H, :nr],
                                           func=AF.Identity, bias=bal[:, 0:1])

                  def emit_gz_chunk(c):
                      lo = c * 128
                      nr = min(128, NAp - lo)
                      gather_chunk(gtab, gid, lo, nr, geT16, 0)
                      ncols = min(NA - lo, nr) if lo < NA else 0
                      if ncols <= 0:
                          return
                      for g in range(3):
                          ps = ppp.tile([128, 128], F32, tag="pp", space="PSUM")
                          nc.tensor.matmul(out=ps[:H, :ncols],
                                           lhsT=wwih[:, g * H:(g + 1) * H],
                                           rhs=geT16[:, lo:lo + ncols],
                                           start=True, stop=True)
                          nc.scalar.activation(
                              out=wgpre3[:, g * NA + lo:g * NA + lo + ncols],
                              in_=ps[:H, :ncols], func=AF.Identity,
                              bias=bw3[:, g:g + 1])

                  def ensure_x(upto_step):
                      while cur["x"] * 128 < min(upto_step + 1, t_run):
                          emit_x_chunk(cur["x"])
                          cur["x"] += 1

                  def ensure_gz(upto_col):
                      while cur["gz"] * 128 < min(upto_col, NAp) \
                              and cur["gz"] * 128 < NAp:
                          emit_gz_chunk(cur["gz"])
                          cur["gz"] += 1

                  def preload_late(jn):
                      sd = steps[jn]
                      r = sd["r"] if sd["wordstep"] else 0
                      PL = plp.tile([128, 3 * r_max + r_max], F32, tag="PL",
                                    space="PSUM")
                      offL = sd["off"] + sd["ne"]
                      if r > 0:
                          nc.tensor.matmul(
                              out=PL[:, 0:3 * r].rearrange("p (g n) -> p g n", g=3),
                              lhsT=ident16[:], rhs=wg3v[:, :, offL:offL + r],
                              start=True, stop=False)
                      PC = pcp.tile([128, 3], F32, tag="PC", space="PSUM")
                      nc.tensor.matmul(
                          out=PC[:, 0:3], lhsT=ident16[:],
                          rhs=xpre16[:, 3 * jn:3 * jn + 3], start=True, stop=False)
                      carry[("PL", jn)] = PL
                      carry[("PC", jn)] = PC

                  def emit_early(jn):
                      sd = steps[jn]
                      ne = sd["ne"]
                      if not sd["wordstep"] or ne == 0:
                          return
                      off = sd["off"]
                      PEa = pep.tile([128, 3 * ne_max], F32, tag="PEa", space="PSUM")
                      nc.tensor.matmul(
                          out=PEa[:, 0:3 * ne].rearrange("p (g n) -> p g n", g=3),
                          lhsT=ident16[:], rhs=wg3v[:, :, off:off + ne],
                          start=True, stop=False)
                      nmm = 3 * len(sd["eblocks"])
                      im = 0
                      for g in range(3):
                          bo = 0
                          for (d_hi, w, dup) in sd["eblocks"]:
                              im += 1
                              nc.tensor.matmul(
                                  out=PEa[:, g * ne + bo:g * ne + bo + w],
                                  lhsT=wwhh[:, g * H:(g + 1) * H],
                                  rhs=Hh[:, jn - d_hi:jn - d_hi + w],
                                  start=False, stop=(im == nmm))
                              bo += w
                      carry[("PEa", jn)] = PEa

                  def emit_early_act(jn):
                      sd = steps[jn]
                      ne = sd["ne"]
                      if not sd["wordstep"] or ne == 0:
                          return
                      PEa = carry.pop(("PEa", jn))
                      TWE = wk.tile([128, 4 * ne_max], F32, tag="TWE")
                      nc.scalar.activation(out=TWE[:, 0:3 * ne], in_=PEa[:, 0:3 * ne],
                                           func=AF.Tanh)
                      carry[("TWE", jn)] = TWE

                  def emit_early_dve(jn):
                      sd = steps[jn]
                      ne = sd["ne"]
                      if not sd["wordstep"] or ne == 0:
                          return
                      TWE = carry[("TWE", jn)]
                      M1E = wk.tile([128, ne_max], MMDT, tag="M1E")
                      nc.vector.scalar_tensor_tensor(
                          out=M1E[:, 0:ne], in0=TWE[:, 0:ne], scalar=1.0,
                          in1=TWE[:, 2 * ne:3 * ne], op0=ALU.add, op1=ALU.mult)
                      M2E = wk.tile([128, ne_max], MMDT, tag="M2E")
                      bo = 0
                      for (d_hi, w, dup) in sd["eblocks"]:
                          nc.vector.scalar_tensor_tensor(
                              out=M2E[:, bo:bo + w], in0=TWE[:, ne + bo:ne + bo + w],
                              scalar=1.0, in1=Cc[:, jn - d_hi:jn - d_hi + w],
                              op0=ALU.add, op1=ALU.mult)
                          bo += w
                      carry[("M1E", jn)] = M1E
                      carry[("M2E", jn)] = M2E

                  def emit_early_cwf(jn):
                      sd = steps[jn]
                      ne = sd["ne"]
                      if not sd["wordstep"] or ne == 0:
                          return
                      off = sd["off"]
                      M1E, M2E = carry[("M1E", jn)], carry[("M2E", jn)]
                      CWFE = wk.tile([128, ne_max], F32, tag="CWFE")
                      nc.vector.tensor_tensor(out=CWFE[:, 0:ne], in0=M1E[:, 0:ne],
                                              in1=M2E[:, 0:ne], op=ALU.add)
                      MCWE = wk.tile([128, ne_max], F32, tag="MCWE")
                      nc.vector.tensor_tensor(out=MCWE[:, 0:ne], in0=CWFE[:, 0:ne],
                                              in1=hm[:, off:off + ne], op=ALU.mult)
                      carry[("MCWE", jn)] = MCWE

                  def emit_early_alpha_mm(jn):
                      sd = steps[jn]
                      ne = sd["ne"]
                      if not sd["wordstep"] or ne == 0:
                          return
                      M1E, M2E = carry.pop(("M1E", jn)), carry.pop(("M2E", jn))
                      PALE = palp.tile([128, ne_max], F32, tag="PALE", space="PSUM")
                      nc.tensor.matmul(out=PALE[:, 0:ne], lhsT=wahh[:],
                                       rhs=M1E[:, 0:ne], start=True, stop=False)
                      nc.tensor.matmul(out=PALE[:, 0:ne], lhsT=wahh[:],
                                       rhs=M2E[:, 0:ne], start=False, stop=True)
                      carry[("PALE", jn)] = PALE

                  def emit_early_alpha_act(jn):
                      sd = steps[jn]
                      ne = sd["ne"]
                      if not sd["wordstep"] or ne == 0:
                          return
                      PALE = carry.pop(("PALE", jn))
                      TWE = carry[("TWE", jn)]
                      nc.scalar.activation(out=TWE[:, 3 * ne:4 * ne], in_=PALE[:, 0:ne],
                                           func=AF.Tanh, bias=apre[:, jn:jn + 1])

                  def emit_early_ee(jn):
                      sd = steps[jn]
                      ne = sd["ne"]
                      if not sd["wordstep"] or ne == 0:
                          return
                      TWE = carry.pop(("TWE", jn))
                      EEE = wk.tile([128, ne_max], F32, tag="EEE")
                      nc.scalar.activation(out=EEE[:, 0:ne], in_=TWE[:, 3 * ne:4 * ne],
                                           func=AF.Exp, scale=0.5, bias=half[:, 0:1])
                      carry[("EEE", jn)] = EEE

                  def emit_early_sums(jn):
                      sd = steps[jn]
                      ne = sd["ne"]
                      if sd["wordstep"] and ne > 0:
                          off = sd["off"]
                          EEE = carry.pop(("EEE", jn))
                          MCWE = carry.pop(("MCWE", jn))
                          WEO = wk.tile([128, ne_max], F32, tag="WEO")
                          nc.vector.scalar_tensor_tensor(
                              out=WEO[:, 0:ne], in0=EEE[:, 0:ne], scalar=1.0,
                              in1=mft[:, off:off + ne], op0=ALU.bypass,
                              op1=ALU.mult, accum_out=EEL3[:, 0:1])
                          WCO = wk.tile([128, ne_max], F32, tag="WCO")
                          nc.vector.scalar_tensor_tensor(
                              out=WCO[:, 0:ne], in0=EEE[:, 0:ne], scalar=1.0,
                              in1=MCWE[:, 0:ne], op0=ALU.bypass,
                              op1=ALU.mult, accum_out=EEL3[:, 1:2])
                      elif sd["wordstep"]:
                          # no early columns: zero the seed slots
                          nc.gpsimd.memset(EEL3[:, 0:2], 0.0)

                  # ---- warm-up: cover steps 0..3 ----
                  ensure_x(4)
                  ensure_gz(off_end(4))

                  # ---- j = 0 ----
                  th0 = wk.tile([H, 3], F32, tag="TWL")
                  nc.scalar.activation(out=th0[:], in_=xpre16[:, 0:3], func=AF.Tanh)
                  c2 = wk.tile([H, 1], F32, tag="c2")
                  nc.vector.scalar_tensor_tensor(
                      out=c2[:], in0=th0[:, 2:3], scalar=1.0, in1=th0[:, 1:2],
                      op0=ALU.add, op1=ALU.mult)
                  nc.vector.tensor_scalar(
                      out=Cc[:, 0:1], in0=c2[:], scalar1=0.5, scalar2=None,
                      op0=ALU.mult)
                  tcn0 = wk.tile([H, 1], F32, tag="tc")
                  nc.scalar.activation(out=tcn0[:], in_=Cc[:, 0:1], func=AF.Tanh)
                  nc.vector.scalar_tensor_tensor(
                      out=Hh[:, 0:1], in0=th0[:, 0:1], scalar=1.0, in1=tcn0[:],
                      op0=ALU.add, op1=ALU.mult)
                  if t_run > 1:
                      preload_late(1)
                      emit_early(1)
                      emit_early_act(1)
                      emit_early_dve(1)
                      emit_early_cwf(1)
                      emit_early_alpha_mm(1)
                      emit_early_alpha_act(1)
                      emit_early_ee(1)
                      emit_early_sums(1)

                  for j in range(1, t_run):
                      sd = steps[j]
                      nxt = steps[j + 1] if j + 1 < t_run else None
                      r = sd["r"] if sd["wordstep"] else 0
                      ws = sd["wordstep"]
                      PL = carry.pop(("PL", j))
                      PC = carry.pop(("PC", j))
                      c_prev = Cc[:, j - 1:j]

                      # --- PE: late matmuls (word first: they feed the chain) ---
                      if ws and r > 0:
                          for g in range(3):
                              for q in range(r):
                                  nc.tensor.matmul(out=PL[:, g * r + q:g * r + q + 1],
                                                   lhsT=wwhh[:, g * H:(g + 1) * H],
                                                   rhs=Hh[:, j - 1:j], start=False,
                                                   stop=(g == 2 and q == r - 1))
                      for g in range(3):
                          nc.tensor.matmul(out=PC[:, g:g + 1],
                                           lhsT=whh[:, g * H:(g + 1) * H],
                                           rhs=Hh[:, j - 1:j], start=False,
                                           stop=(g == 2))

                      # --- PE: preloads + early matmuls for j+1 ---
                      if nxt is not None:
                          preload_late(j + 1)
                          emit_early(j + 1)

                      # --- ACT: tanh late: word block (chain), then char ---
                      TWL = wk.tile([128, 3 * r_max + 3 + r_max], F32, tag="TWL")
                      if ws and r > 0:
                          nc.scalar.activation(out=TWL[:, 0:3 * r],
                                               in_=PL[:, 0:3 * r], func=AF.Tanh)
                      nc.scalar.activation(out=TWL[:, 3 * r:3 * r + 3],
                                           in_=PC[:, 0:3], func=AF.Tanh)
                      if nxt is not None:
                          emit_early_act(j + 1)

                      # --- DVE: late m1/m2/cw2 ---
                      if ws and r > 0:
                          M1L = wk.tile([128, r_max], MMDT, tag="M1L")
                          nc.vector.scalar_tensor_tensor(
                              out=M1L[:, 0:r], in0=TWL[:, 0:r], scalar=1.0,
                              in1=TWL[:, 2 * r:3 * r], op0=ALU.add, op1=ALU.mult)
                          M2L = wk.tile([128, r_max], MMDT, tag="M2L")
                          for q in range(r):
                              nc.vector.scalar_tensor_tensor(
                                  out=M2L[:, q:q + 1], in0=TWL[:, r + q:r + q + 1],
                                  scalar=1.0, in1=c_prev, op0=ALU.add, op1=ALU.mult)
                          CW2 = wk.tile([128, r_max], MMDT, tag="CW2")
                          nc.vector.tensor_tensor(out=CW2[:, 0:r], in0=M1L[:, 0:r],
                                                  in1=M2L[:, 0:r], op=ALU.add)
                      if sd["blend"] or not ws:
                          DD = wk.tile([H, 1], F32, tag="DD")
                          nc.vector.tensor_tensor(out=DD[:], in0=TWL[:, 3 * r + 1:3 * r + 2],
                                                  in1=c_prev, op=ALU.subtract)
                          E2 = wk.tile([H, 1], F32, tag="E2")
                          nc.vector.scalar_tensor_tensor(
                              out=E2[:], in0=TWL[:, 3 * r + 2:3 * r + 3], scalar=1.0,
                              in1=DD[:], op0=ALU.add, op1=ALU.mult)
                          CCPL = wk.tile([H, 1], F32, tag="CCPL")
                          nc.vector.scalar_tensor_tensor(
                              out=CCPL[:], in0=E2[:], scalar=0.5, in1=c_prev,
                              op0=ALU.mult, op1=ALU.add)
                      if nxt is not None:
                          emit_early_dve(j + 1)

                      # --- DVE: t_g copy + late mcw into MCN ---
                      if ws:
                          nc.vector.tensor_copy(out=MCN[:, 1:2],
                                                in_=TWL[:, 3 * r + 1:3 * r + 2])
                          if r > 0:
                              offL = sd["off"] + sd["ne"]
                              nc.vector.tensor_tensor(
                                  out=MCN[:, 2:2 + r], in0=CW2[:, 0:r],
                                  in1=hm[:, offL:offL + r], op=ALU.mult)
                      if nxt is not None:
                          emit_early_cwf(j + 1)

                      # --- PE: late alpha mm + early alpha mms ---
                      if ws and r > 0:
                          nc.tensor.matmul(out=PL[:, 3 * r:3 * r + r],
                                           lhsT=wahh[:], rhs=CW2[:, 0:r],
                                           start=True, stop=True)
                      if nxt is not None:
                          emit_early_alpha_mm(j + 1)

                      if ws:
                          # --- ACT: late talpha + ee (into EEL3) ---
                          if r > 0:
                              nc.scalar.activation(
                                  out=TWL[:, 3 * r + 3:3 * r + 3 + r],
                                  in_=PL[:, 3 * r:3 * r + r],
                                  func=AF.Tanh, bias=apre[:, j:j + 1])
                          nc.scalar.activation(
                              out=EEL3[:, 2:3 + r], in_=TWL[:, 3 * r + 2:3 * r + 3 + r],
                              func=AF.Exp, scale=0.5, bias=half[:, 0:1])
                          if nxt is not None:
                              emit_early_alpha_act(j + 1)

                          # --- DVE: seed-folded den/num ---
                          lmo = lm_offs[j]
                          DEN = ak.tile([128, 1], F32, tag="DEN")
                          WLO = wk.tile([128, 3 + r_max], F32, tag="WLO")
                          nc.vector.scalar_tensor_tensor(
                              out=WLO[:, 0:3 + r], in0=EEL3[:, 0:3 + r], scalar=1.0,
                              in1=mld[:, lmo:lmo + 3 + r], op0=ALU.bypass,
                              op1=ALU.mult, accum_out=DEN[:])
                          NUM = ak.tile([128, 1], F32, tag="NUM")
                          WLC = wk.tile([128, 3 + r_max], F32, tag="WLC")
                          nc.vector.scalar_tensor_tensor(
                              out=WLC[:, 0:2 + r], in0=EEL3[:, 1:3 + r], scalar=1.0,
                              in1=MCN[:, 0:2 + r], op0=ALU.bypass,
                              op1=ALU.mult, accum_out=NUM[:])
                          if sd["blend"]:
                              hwc = sd["hw_col"]
                              DEN2 = ak.tile([128, 1], F32, tag="DEN")
                              nc.vector.tensor_tensor(out=DEN2[:], in0=DEN[:],
                                                      in1=zct[:, hwc:hwc + 1],
                                                      op=ALU.add)
                              NUM2 = ak.tile([128, 1], F32, tag="NUM")
                              nc.vector.scalar_tensor_tensor(
                                  out=NUM2[:], in0=CCPL[:],
                                  scalar=zct[:, hwc:hwc + 1], in1=NUM[:],
                                  op0=ALU.mult, op1=ALU.add)
                              DEN, NUM = DEN2, NUM2
                          RCP = ak.tile([128, 1], F32, tag="RCP")
                          nc.vector.reciprocal(out=RCP[:], in_=DEN[:])

                          # --- ACT: tcn ---
                          TCN = ak.tile([128, 1], F32, tag="TCN")
                          nc.scalar.activation(out=TCN[:], in_=NUM[:], func=AF.Tanh,
                                               scale=RCP[:, 0:1])
                          nc.vector.tensor_tensor(out=Cc[:, j:j + 1], in0=NUM[:],
                                                  in1=RCP[:], op=ALU.mult)
                          # --- DVE: h write ---
                          nc.vector.scalar_tensor_tensor(
                              out=Hh[:, j:j + 1], in0=TWL[:, 3 * r:3 * r + 1],
                              scalar=1.0, in1=TCN[:], op0=ALU.add, op1=ALU.mult)
                      else:
                          if nxt is not None:
                              emit_early_alpha_act(j + 1)
                          nc.vector.tensor_copy(out=Cc[:, j:j + 1], in_=CCPL[:])
                          TCN = ak.tile([128, 1], F32, tag="TCN")
                          nc.scalar.activation(out=TCN[:], in_=Cc[:, j:j + 1],
                                               func=AF.Tanh)
                          nc.vector.scalar_tensor_tensor(
                              out=Hh[:, j:j + 1], in0=TWL[:, 3 * r:3 * r + 1],
                              scalar=1.0, in1=TCN[:], op0=ALU.add, op1=ALU.mult)

                      if nxt is not None:
                          emit_early_ee(j + 1)
                          emit_early_sums(j + 1)
                      # --- interleaved prestage chunks (just-in-time) ---
                      ensure_x(j + 6)
                      ensure_gz(off_end(j + 4))

              # -------------- epilogue: tag head + argmax --------------
              with tc.tile_pool(name="spsum", bufs=2, space="PSUM") as sp:
                nchunks = (t_run + 127) // 128
                for c in range(nchunks):
                    lo = c * 128
                    nr = min(128, t_run - lo)
                    pt = sp.tile([128, NL], F32, tag="pt", space="PSUM")
                    nc.tensor.matmul(out=pt[:nr], lhsT=Hh[:, lo:lo + nr],
                                     rhs=wtag[:], start=True, stop=True)
                    lg = wk.tile([128, NL], F32, tag="lg")
                    nc.vector.tensor_tensor(out=lg[:nr], in0=pt[:nr], in1=btg[:nr],
                                            op=ALU.add)
                    mx = wk.tile([128, 1], F32, tag="mx")
                    nc.vector.tensor_reduce(out=mx[:nr], in_=lg[:nr], axis=AX.X,
                                            op=ALU.max)
                    eq = wk.tile([128, NL], F32, tag="eq")
                    nc.vector.tensor_scalar(out=eq[:nr], in0=lg[:nr],
                                            scalar1=mx[:nr, 0:1], scalar2=None,
                                            op0=ALU.is_equal)
                    j2 = wk.tile([128, NL], F32, tag="j2")
                    im = wk.tile([128, 1], F32, tag="im")
                    nc.vector.tensor_tensor(out=j2[:nr], in0=eq[:nr], in1=iot[:nr],
                                            op=ALU.mult)
                    nc.vector.tensor_reduce(out=im[:nr], in_=j2[:nr], axis=AX.X,
                                            op=ALU.min)
                    tf = wk.tile([128, 1], F32, tag="tf")
                    nc.vector.scalar_tensor_tensor(
                        out=tf[:nr], in0=im[:nr], scalar=1e4, in1=mTt[:nr, c:c + 1],
                        op0=ALU.add, op1=ALU.mult)
                    ti = wk.tile([128, 1], I32, tag="ti")
                    nc.vector.tensor_copy(out=ti[:nr], in_=tf[:nr])
                    nc.sync.dma_start(out=tags[lo:lo + nr, None], in_=ti[:nr])

    return nc


def make_in_maps(inputs, steps, NA, NB, t_run=T):
    sh = prep_shared(inputs, t_run)
    NAp = max(128, ((NA + 127) // 128) * 128)
    in_maps = []
    mask_in = np.asarray(inputs["mask"])
    for b in range(B):
        gid, mft, hm, mld, zc, LM3 = pack_core(
            b, steps, inputs["gaz_word_ids"], inputs["gaz_starts"],
            inputs["gaz_mask"], NA, NB, t_run)
        gidp = np.zeros(NAp, np.int32)
        gidp[:max(NA, 1)] = gid[:max(NA, 1)] if NA > 0 else 0
        nch = max(1, (t_run + 127) // 128)
        mT = np.zeros((H, nch), np.float32)
        mrow = mask_in[b, :t_run].astype(np.float32)
        for c in range((t_run + 127) // 128):
            nr = min(128, t_run - c * 128)
            mT[:nr, c] = mrow[c * 128:c * 128 + nr]
        m = dict(sh)
        m["wid"] = np.asarray(inputs["word_inputs"])[b, :t_run].astype(np.int32).copy()
        m["bid"] = np.asarray(inputs["biword_inputs"])[b, :t_run].astype(np.int32).copy()
        m["gid"] = gidp
        m["mft"] = np.ascontiguousarray(np.broadcast_to(mft[None, :], (H, max(NA, 1))))
        m["hm"] = np.ascontiguousarray(np.broadcast_to(hm[None, :], (H, max(NA, 1))))
        m["mld"] = np.ascontiguousarray(np.broadcast_to(mld[None, :], (H, max(LM3, 1))))
        m["zc"] = np.ascontiguousarray(np.broadcast_to(zc[None, :], (H, max(NB, 1))))
        m["maskT"] = mT
        in_maps.append(m)
    return in_maps


def kernel(**inputs) -> np.ndarray:
    steps, NA, NB = build_structure(inputs["gaz_starts"], inputs["gaz_mask"], T)
    nc = build_nc(steps, NA, NB, T)
    _legalize_single_wait(nc)
    mybir.codegen_inst_isa_subclasses(nc)
    in_maps = make_in_maps(inputs, steps, NA, NB, T)
    res = run_bass_kernel_spmd(nc, in_maps, list(range(B)))
    out = np.stack([res.results[b]["tags"] for b in range(B)], axis=0)
    return out.astype(np.int32)
